# revision 1
# baseline (speedup 1.0000x reference)
"""GCNConv (PyG-faithful, normalize=True, add_self_loops=True) on 8 Trainium2
NeuronCores via Bass/Tile.

Strategy (1D graph/data parallel):
  - Nodes are partitioned across the 8 cores (12500 rows each, padded to
    12544 = 98 blocks of 128).
  - Phase A: each core computes h_k = x_k @ W (fp32 matmuls), scales rows by
    dinv (symmetric GCN normalization, computed host-side from the edge
    index), casts to bf16 and AllGathers the scaled table
    g = dinv[:,None] * (x @ W) into every core's DRAM.
  - Phase B: each core owns 1/8 of the destination nodes. Edges (including
    self-loops) are host-sorted by destination block; per 128-edge tile a
    dma_gather (SWDGE, 4 queues round-robin) fetches g[src] rows (bf16), a
    host-precomputed one-hot selection tile (fp8, streamed from DRAM via
    HWDGE) feeds a TensorE matmul that segment-sums messages into a
    per-block PSUM accumulator. The epilogue scales by dinv_dst on ScalarE
    (PSUM->SBUF copy), adds bias per window on VectorE, and stores one
    window (896 nodes) per DMA.

  Per-(block, chunk) tile counts are computed from the actual edge data at
  call time (the program is compiled per call), maxed across cores so all 8
  cores run an identical (SPMD) program.
"""

import sys

if "/opt/trn_rl_repo" not in sys.path:
    sys.path.insert(0, "/opt/trn_rl_repo")

import numpy as np

P = 128          # partitions / tile edge count / feature dim
NCORES = 8
WBLK = 7         # blocks per window
CHUNKS = 4       # src chunks for int16 gather indices

_PAD_DL = 300    # sentinel dst_local for pad edges -> all-zero sel column


def _pack(x, edge_index, weight, b):
    """Host-side preprocessing: sharding, normalization metadata, gather
    index packing, one-hot sel tiles. All numpy, vectorized."""
    import ml_dtypes

    bias = b
    x = np.ascontiguousarray(np.asarray(x, dtype=np.float32))
    ei = np.asarray(edge_index)
    weight = np.ascontiguousarray(np.asarray(weight, dtype=np.float32))
    bias = np.asarray(bias, dtype=np.float32).reshape(-1)

    n, nin = x.shape
    nout = weight.shape[1]
    assert nin == P and nout == P, (nin, nout)
    assert n % NCORES == 0, n
    nb = n // NCORES                      # nodes per core (12500)
    blocks = (nb + P - 1) // P            # blocks per core (98)
    nbp = blocks * P                      # padded nodes per core (12544)
    npad = nbp * NCORES                   # padded table rows (100352)
    wblk = WBLK if blocks % WBLK == 0 else 1
    nwin = blocks // wblk                 # windows (14)
    chunk_rows = npad // CHUNKS           # rows per chunk (25088)
    assert chunk_rows < 32768, chunk_rows

    src = ei[0].astype(np.int64)
    dst = ei[1].astype(np.int64)

    deg = np.bincount(dst, minlength=n).astype(np.float32) + 1.0
    dinv = 1.0 / np.sqrt(deg)

    loop = np.arange(n, dtype=np.int64)
    src_a = np.concatenate([src, loop])
    dst_a = np.concatenate([dst, loop])
    m = src_a.shape[0]

    core = dst_a // nb
    dlc = dst_a - core * nb               # dst local to core
    blk = dlc >> 7
    dl = (dlc & 127).astype(np.int64)
    grow = (src_a // nb) * nbp + (src_a % nb)   # padded global row of src
    chunk = grow // chunk_rows
    rel = (grow % chunk_rows).astype(np.int16)

    key = (core * blocks + blk) * CHUNKS + chunk
    order = np.argsort(key, kind="stable")
    karr = core[order]
    relarr = rel[order]
    dlarr = dl[order]
    gkey = key[order]

    counts = np.bincount(key, minlength=NCORES * blocks * CHUNKS).reshape(
        NCORES, blocks, CHUNKS
    )
    t_bc = -(-counts.max(axis=0) // P)    # [blocks, CHUNKS] tiles per slot

    # global tile layout: for w in windows: for c in chunks: for b in window
    tile_off = np.zeros((blocks, CHUNKS), np.int64)
    wbase = np.zeros(nwin + 1, np.int64)
    col = 0
    for w in range(nwin):
        wbase[w] = col
        for c in range(CHUNKS):
            for bb in range(w * wblk, (w + 1) * wblk):
                tile_off[bb, c] = col
                col += t_bc[bb, c]
    t_total = int(col)
    wbase[nwin] = col

    # scatter edges into per-core packed arrays
    gs = np.zeros(NCORES * blocks * CHUNKS, np.int64)
    gs[1:] = np.cumsum(counts.ravel())[:-1]
    rank = np.arange(m, dtype=np.int64) - gs[gkey]
    base_flat = (tile_off * P).ravel()    # same for all cores
    dest = base_flat[(gkey % (blocks * CHUNKS))] + rank

    idx_lin = np.zeros((NCORES, t_total * P), np.int16)
    dl_lin = np.full((NCORES, t_total * P), _PAD_DL, np.int16)
    idx_lin[karr, dest] = relarr
    dl_lin[karr, dest] = dlarr.astype(np.int16)

    # wrap-16 + replicate to 128 partitions for dma_gather idx layout
    l16 = t_total * P // 16
    idx_w = idx_lin.reshape(NCORES, l16, 16).transpose(0, 2, 1)  # [8,16,L16]
    idx_pack = np.ascontiguousarray(np.tile(idx_w, (1, NCORES, 1)))  # [8,128,L16]

    # host-precomputed one-hot sel tiles, fp8: sel[e, gt, d] = (dl[gt,e] == d)
    sel_pack = np.empty((NCORES, P, t_total * P), ml_dtypes.float8_e4m3)
    dgrid = np.arange(P, dtype=np.int16)[None, None, :]
    for k in range(NCORES):
        dlr = dl_lin[k].reshape(t_total, P)          # [gt, e]
        sel_k = dlr.T[:, :, None] == dgrid           # [e, gt, d] bool
        sel_pack[k] = sel_k.reshape(P, t_total * P).astype(ml_dtypes.float8_e4m3)

    # per-core xT, dinv
    xt = np.zeros((NCORES, P, nbp), np.float32)
    dinv_t = np.zeros((NCORES, P, blocks), np.float32)
    for k in range(NCORES):
        xs = x[k * nb : (k + 1) * nb]
        xt[k, :, :nb] = xs.T
        dv = np.zeros(nbp, np.float32)
        dv[:nb] = dinv[k * nb : (k + 1) * nb]
        dinv_t[k] = dv.reshape(blocks, P).T
    bias_rep = np.ascontiguousarray(np.tile(bias[None, :], (P, 1)))

    meta = dict(
        n=n, nb=nb, blocks=blocks, nbp=nbp, npad=npad, nwin=nwin, wblk=wblk,
        chunk_rows=chunk_rows, t_bc=t_bc, tile_off=tile_off,
        wbase=wbase, t_total=t_total, l16=l16,
    )
    in_maps = [
        {
            "xt": xt[k],
            "w_in": weight,
            "bias": bias_rep,
            "dinv": dinv_t[k],
            "idxp": idx_pack[k],
            "selp": sel_pack[k],
        }
        for k in range(NCORES)
    ]
    return meta, in_maps


def _build_program(meta):
    from concourse import bass, bacc, mybir
    import concourse.tile as tile

    blocks = meta["blocks"]
    nbp = meta["nbp"]
    npad = meta["npad"]
    nwin = meta["nwin"]
    wblk = meta["wblk"]
    chunk_rows = meta["chunk_rows"]
    t_bc = meta["t_bc"]
    tile_off = meta["tile_off"]
    wbase = meta["wbase"]
    t_total = meta["t_total"]
    l16 = meta["l16"]
    jmax = int((wbase[1:] - wbase[:-1]).max())
    selmax = int(t_bc.sum(axis=1).max())  # max tiles per block

    f32 = mybir.dt.float32
    bf16 = mybir.dt.bfloat16
    fp8 = mybir.dt.float8e4

    nc = bacc.Bacc(num_swdge_queues=4)
    xt_in = nc.declare_dram_parameter("xt", [P, nbp], f32, isOutput=False)
    w_in = nc.declare_dram_parameter("w_in", [P, P], f32, isOutput=False)
    bias_in = nc.declare_dram_parameter("bias", [P, P], f32, isOutput=False)
    dinv_in = nc.declare_dram_parameter("dinv", [P, blocks], f32, isOutput=False)
    idx_in = nc.declare_dram_parameter("idxp", [P, l16], mybir.dt.int16, isOutput=False)
    sel_in = nc.declare_dram_parameter("selp", [P, t_total * P], fp8, isOutput=False)
    out_ext = nc.declare_dram_parameter("out", [nbp, P], f32, isOutput=True)

    h_shard = nc.dram_tensor("h_shard", [nbp, P], bf16)
    g_table = nc.dram_tensor("g_table", [npad, P], bf16, addr_space="Shared")

    with tile.TileContext(nc) as tc:
        with (
            tc.tile_pool(name="const", bufs=1) as cpool,
            tc.tile_pool(name="work", bufs=4) as wpool,
            tc.tile_pool(name="msgp", bufs=2) as mpool,
            tc.tile_pool(name="selp", bufs=6) as spool,
            tc.tile_pool(name="outp", bufs=2) as opool,
            tc.tile_pool(name="psA", bufs=2, space="PSUM") as psA,
            tc.tile_pool(name="psB", bufs=4, space="PSUM") as psB,
        ):
            # constants / metadata loads
            w_sb = cpool.tile([P, P], f32, tag="w")
            nc.sync.dma_start(out=w_sb[:], in_=w_in[:])
            bias_sb = cpool.tile([P, P], f32, tag="bias")
            nc.sync.dma_start(out=bias_sb[:], in_=bias_in[:])
            dinv_sb = cpool.tile([P, blocks], f32, tag="dinv")
            nc.sync.dma_start(out=dinv_sb[:], in_=dinv_in[:])
            idx_sb = cpool.tile([P, l16], mybir.dt.int16, tag="idx")
            for i in range(4):
                s = l16 // 4
                e = l16 if i == 3 else (i + 1) * s
                nc.sync.dma_start(out=idx_sb[:, i * s : e], in_=idx_in[:, i * s : e])

            # ---- phase A: h = x @ W, scale by dinv, cast bf16, allgather
            nchunk = next(d for d in (7, 8, 4, 2, 1) if blocks % d == 0)
            cw = nbp // nchunk            # nodes per chunk (1792 full-size)
            tpc = cw // P                 # tiles per chunk
            for ch in range(nchunk):
                xt_t = wpool.tile([P, cw], f32, tag="xt")
                nc.sync.dma_start(out=xt_t[:], in_=xt_in[:, ch * cw : (ch + 1) * cw])
                hbig = wpool.tile([P, tpc, P], bf16, tag="hbig")
                for t in range(tpc):
                    ph = psA.tile([P, P], f32, tag="ph")
                    nc.tensor.matmul(
                        out=ph[:],
                        lhsT=xt_t[:, t * P : (t + 1) * P],
                        rhs=w_sb[:],
                        start=True,
                        stop=True,
                    )
                    gb = ch * tpc + t
                    nc.vector.tensor_scalar(
                        out=hbig[:, t, :],
                        in0=ph[:],
                        scalar1=dinv_sb[:, gb : gb + 1],
                        scalar2=None,
                        op0=mybir.AluOpType.mult,
                    )
                nc.sync.dma_start(
                    out=h_shard[ch * cw : (ch + 1) * cw, :].rearrange(
                        "(t p) f -> p t f", p=P
                    ),
                    in_=hbig[:],
                )

            nc.gpsimd.collective_compute(
                "AllGather",
                mybir.AluOpType.bypass,
                replica_groups=[list(range(NCORES))],
                ins=[h_shard[:]],
                outs=[g_table[:]],
            )

            # ---- phase B: gather + one-hot segment matmul per dst block
            secmax = 0
            for w in range(nwin):
                for c in range(CHUNKS):
                    secmax = max(
                        secmax,
                        int(t_bc[w * wblk : (w + 1) * wblk, c].sum()),
                    )
            for w in range(nwin):
                msg = mpool.tile([P, jmax, P], bf16, tag="msg")
                sec0s = [0] * CHUNKS
                sels = [None] * CHUNKS
                for c in range(CHUNKS):
                    sec0 = None
                    seclen = 0
                    for bb in range(w * wblk, (w + 1) * wblk):
                        if t_bc[bb, c] > 0:
                            if sec0 is None:
                                sec0 = int(tile_off[bb, c])
                            seclen += int(t_bc[bb, c])
                    if seclen == 0:
                        continue
                    sec0s[c] = sec0
                    lo = sec0 - int(wbase[w])
                    nc.gpsimd.dma_gather(
                        out_ap=msg[:, lo : lo + seclen, :],
                        in_ap=g_table[c * chunk_rows : (c + 1) * chunk_rows, :],
                        idxs_ap=idx_sb[:, sec0 * 8 : (sec0 + seclen) * 8],
                        num_idxs=seclen * P,
                        num_idxs_reg=seclen * P,
                        elem_size=P,
                        single_packet=False,
                        queue_num=c,
                    )
                    # one sel DMA per section, on ScalarE's HWDGE queue
                    selw = spool.tile([P, secmax * P], fp8, tag="selw")
                    nc.scalar.dma_start(
                        out=selw[:, : seclen * P],
                        in_=sel_in[:, sec0 * P : (sec0 + seclen) * P],
                    )
                    sels[c] = selw
                osb_w = opool.tile([P, wblk, P], f32, tag="osbw")
                for j, bb in enumerate(range(w * wblk, (w + 1) * wblk)):
                    ntiles = int(t_bc[bb].sum())
                    assert ntiles > 0
                    acc = psB.tile([P, P], f32, tag="acc")
                    ti = 0
                    for c in range(CHUNKS):
                        tb = int(t_bc[bb, c])
                        for t in range(tb):
                            gt = int(tile_off[bb, c]) + t
                            mcol = gt - int(wbase[w])
                            st = gt - sec0s[c]
                            nc.tensor.matmul(
                                out=acc[:],
                                lhsT=sels[c][:, st * P : (st + 1) * P],
                                rhs=msg[:, mcol, :],
                                start=(ti == 0),
                                stop=(ti == ntiles - 1),
                            )
                            ti += 1
                    # epilogue: scale by dinv_dst on ScalarE (PSUM -> SBUF)
                    nc.scalar.activation(
                        out=osb_w[:, j, :],
                        in_=acc[:],
                        func=mybir.ActivationFunctionType.Copy,
                        scale=dinv_sb[:, bb : bb + 1],
                    )
                # bias add for the whole window on VectorE, then store
                nc.vector.tensor_tensor(
                    out=osb_w[:],
                    in0=osb_w[:],
                    in1=bias_sb[:].unsqueeze(1).to_broadcast([P, wblk, P]),
                    op=mybir.AluOpType.add,
                )
                nc.sync.dma_start(
                    out=out_ext[w * wblk * P : (w + 1) * wblk * P, :].rearrange(
                        "(j p) f -> p j f", p=P
                    ),
                    in_=osb_w[:],
                )

    nc.finalize()
    return nc


def _run(inputs, trace=False, trace_cores=None):
    from concourse.bass_utils import run_bass_kernel_spmd

    meta, in_maps = _pack(**inputs)
    nc = _build_program(meta)
    res = run_bass_kernel_spmd(
        nc,
        in_maps,
        list(range(NCORES)),
        trace=trace,
        trace_cores=trace_cores,
    )
    n, nb, nbp = meta["n"], meta["nb"], meta["nbp"]
    out = np.empty((n, P), np.float32)
    for k in range(NCORES):
        out[k * nb : (k + 1) * nb] = np.asarray(res.results[k]["out"])[:nb]
    return out, res


def kernel(x, edge_index, weight, b):
    out, _ = _run(dict(x=x, edge_index=edge_index, weight=weight, b=b))
    return out


if __name__ == "__main__":
    rng = np.random.default_rng(0)
    n, e = 100000, 1600000
    x = rng.standard_normal((n, P), dtype=np.float32)
    ei = rng.integers(0, n, (2, e)).astype(np.int64)
    w = (rng.standard_normal((P, P)) / np.sqrt(P)).astype(np.float32)
    bb = (rng.standard_normal(P) * 0.02).astype(np.float32)
    out = kernel(x, ei, w, bb)
    print("out", out.shape, out.dtype)



# revision 23
# speedup vs baseline: 1.7880x; 1.7880x over previous
"""GCNConv (PyG-faithful, normalize=True, add_self_loops=True) on 8 Trainium2
NeuronCores via Bass/Tile.

Strategy (1D graph/data parallel, v2):
  - Nodes partitioned across 8 cores (12500 rows each, padded to 12544 = 98
    blocks of 128).
  - Phase A: each core computes h = (x @ W) * dinv_row in bf16 and writes it
    out in four "quarter" pieces (25/25/24/24 blocks). After each quarter an
    AllGather ships that quarter of every core into a per-quarter shared
    table chunk, so phase B can start gathering from chunk 0 while chunks
    1-3 are still in flight.
  - Phase B: each core owns 1/8 of the destinations. Edges (self-loops
    excluded - they are folded into the epilogue) are host-sorted by
    (dst-window, src-quarter, dst-block) and packed densely per
    (window, quarter) section; trailing pad indices are -1, which the SWDGE
    gather ucode trims, so pads cost no DMA packets. Four SWDGE queues (one
    per source quarter) gather 256B bf16 rows; host-precomputed one-hot fp8
    sel tiles feed TensorE matmuls that segment-sum each destination
    block's messages in PSUM. Epilogue per block: add the self-loop term
    (own h row), scale by dinv_dst, add bias.
"""

import sys

if "/opt/trn_rl_repo" not in sys.path:
    sys.path.insert(0, "/opt/trn_rl_repo")

import numpy as np

P = 128
NCORES = 8
WBLK = 7          # dst blocks per window
NW = 14           # windows (98 blocks / 7)
# Source-quarter sizes are skewed: chunk q's gathers cannot start before
# AllGather q completes, and the 4 AllGathers serialize on the CC stream,
# so later chunks get fewer packets to catch up.
Q_BLKS = [26, 25, 24, 23]          # blocks per source quarter
Q_OFF_B = [0, 26, 51, 75]          # quarter start block
N_NODES = 100000
NB = N_NODES // NCORES             # 12500
BLOCKS = 98
NBP = BLOCKS * P                   # 12544


def _pack(x, edge_index, weight, b):
    import ml_dtypes

    x = np.asarray(x, dtype=np.float32)
    ei = np.asarray(edge_index)
    weight = np.asarray(weight, dtype=np.float32)
    bias = np.asarray(b, dtype=np.float32).reshape(-1)

    n, nin = x.shape
    assert n == N_NODES and nin == P and weight.shape == (P, P)
    q_rows = [q * P for q in Q_BLKS]            # [3200,3200,3072,3072]
    q_off_r = [o * P for o in Q_OFF_B]          # row offsets within a shard
    chunk_rows = [NCORES * r for r in q_rows]

    src = ei[0].astype(np.int64)
    dst = ei[1].astype(np.int64)
    m = src.shape[0]

    deg = np.bincount(dst, minlength=n).astype(np.float32) + 1.0
    dinv = 1.0 / np.sqrt(deg)

    # --- per-edge coordinates (no self-loops; folded into epilogue)
    core = dst // NB
    dlc = dst - core * NB
    blk = dlc >> 7                   # dst block within core [0,98)
    dl = (dlc & 127).astype(np.int64)
    w = blk // WBLK                  # window [0,14)

    ks = src // NB
    ls = src - ks * NB
    sb = ls >> 7                     # src block within owner core
    q = np.digitize(sb, Q_OFF_B[1:])             # quarter 0..3
    rel = ks * np.take(q_rows, q) + (ls - np.take(q_off_r, q))
    assert rel.max() < 32768

    # order edges per core by (w, q, blk)
    sect = (w * 4 + q)               # (w,q) section id [0,56)
    key = (core * 56 + sect) * BLOCKS + blk
    order = np.argsort(key, kind="stable")
    karr = core[order]
    relarr = rel[order].astype(np.int16)
    dlarr = dl[order]
    sectarr = sect[order]
    blkarr = blk[order]

    # counts
    cnt_kwqb = np.bincount(
        (core * 56 + sect) * BLOCKS + blk, minlength=NCORES * 56 * BLOCKS
    ).reshape(NCORES, NW, 4, BLOCKS)
    cnt_kwq = cnt_kwqb.sum(axis=3)                       # [8, 14, 4]
    S_wq = -(-cnt_kwq.max(axis=0) // P)                  # [14, 4] tiles
    # per-core block start slot within its (w,q) section
    bstart = np.zeros((NCORES, NW, 4, BLOCKS), np.int64)
    for wi in range(NW):
        bl = slice(wi * WBLK, (wi + 1) * WBLK)
        c = cnt_kwqb[:, wi, :, bl]
        bstart[:, wi, :, bl] = np.cumsum(c, axis=2) - c

    # global tile layout + matmul plan
    sec0 = np.zeros((NW, 4), np.int64)      # global tile offset per section
    O_wq = np.zeros((NW, 4), np.int64)      # msg slot-tile offset in window
    t_total = 0
    for wi in range(NW):
        off = 0
        for qi in range(4):
            sec0[wi, qi] = t_total
            O_wq[wi, qi] = off
            t_total += int(S_wq[wi, qi])
            off += int(S_wq[wi, qi])
    jmax = int(S_wq.sum(axis=1).max())

    # matmul plan: per (w,q) ordered list of (b, t); col index global
    mm_col_of = np.full((NW, 4, BLOCKS, int(S_wq.max()) + 1), -1, np.int64)
    plan = [[[] for _ in range(4)] for _ in range(NW)]   # [(b, t, col)]
    selcol0 = np.zeros((NW, 4), np.int64)
    mm_total = 0
    ntiles_b = np.zeros(BLOCKS, np.int64)
    for wi in range(NW):
        for qi in range(4):
            selcol0[wi, qi] = mm_total
            for bb in range(wi * WBLK, (wi + 1) * WBLK):
                c = cnt_kwqb[:, wi, qi, bb]
                if c.max() == 0:
                    continue
                s = bstart[:, wi, qi, bb]
                e = s + c
                T0 = int((s // P).min())
                T1 = int((-(-e // P)).max())
                for t in range(T0, T1):
                    mm_col_of[wi, qi, bb, t] = mm_total
                    plan[wi][qi].append((bb, t, mm_total))
                    ntiles_b[bb] += 1
                    mm_total += 1
    assert (ntiles_b > 0).all()
    selmax = int(
        max(len(plan[wi][qi]) for wi in range(NW) for qi in range(4))
    )

    # per-core packed arrays
    gs = np.zeros(NCORES * NW * 4 * BLOCKS, np.int64)
    cr = cnt_kwqb.reshape(-1)
    gs[1:] = np.cumsum(cr)[:-1]
    kk = (karr * 56 + sectarr) * BLOCKS + blkarr
    rank_in_b = np.arange(m, dtype=np.int64) - gs[kk]
    wa = sectarr // 4
    qa = sectarr % 4
    slot = (
        bstart[karr, wa, qa, blkarr] + rank_in_b
    )                                                  # slot within section
    tile_in_sec = slot >> 7
    part = slot & 127
    gslot = sec0[wa, qa] * P + slot                    # global slot

    # pad slots gather row 0 of their chunk (sel column is zero, so the
    # value is discarded). Gathering pads costs ~4% extra packets but keeps
    # the per-core descriptor count equal to the static num_idxs, which the
    # SWDGE ring accounting requires (num_idxs_reg must match the actual
    # count, and per-core runtime registers proved unschedulable safely).
    idx_lin = np.zeros((NCORES, t_total * P), np.int16)
    idx_lin[karr, gslot] = relarr

    mmi = mm_col_of[wa, qa, blkarr, tile_in_sec]
    assert (mmi >= 0).all()
    sel_u8 = np.zeros((NCORES, P, mm_total * P), np.uint8)
    sel_u8[karr, part, mmi * P + dlarr] = 0x38         # fp8e4m3 1.0
    sel_pack = sel_u8.view(ml_dtypes.float8_e4m3)

    # wrap-16 + replicate to 128 partitions
    l16 = t_total * P // 16
    idx_w = idx_lin.reshape(NCORES, l16, 16).transpose(0, 2, 1)
    idx_pack = np.ascontiguousarray(np.tile(idx_w, (1, NCORES, 1)))

    # per-core xt (bf16), dinv
    xt = np.zeros((NCORES, P, NBP), ml_dtypes.bfloat16)
    dinv_t = np.zeros((NCORES, P, BLOCKS), np.float32)
    for k in range(NCORES):
        xs = x[k * NB : (k + 1) * NB]
        xt[k, :, :NB] = xs.T.astype(ml_dtypes.bfloat16)
        dv = np.zeros(NBP, np.float32)
        dv[:NB] = dinv[k * NB : (k + 1) * NB]
        dinv_t[k] = dv.reshape(BLOCKS, P).T
    w_bf = np.ascontiguousarray(weight.astype(ml_dtypes.bfloat16))
    bias_rep = np.ascontiguousarray(np.tile(bias[None, :], (P, 1)))
    ident_u8 = np.zeros((P, P), np.uint8)
    ident_u8[np.arange(P), np.arange(P)] = 0x38       # fp8e4m3 identity
    ident = ident_u8.view(ml_dtypes.float8_e4m3)

    meta = dict(
        q_rows=q_rows, chunk_rows=chunk_rows, S_wq=S_wq, sec0=sec0, O_wq=O_wq,
        jmax=jmax, t_total=t_total, l16=l16, plan=plan, selcol0=selcol0,
        mm_total=mm_total, selmax=selmax, ntiles_b=ntiles_b,
    )
    in_maps = [
        {
            "xt": xt[k],
            "w_in": w_bf,
            "bias": bias_rep,
            "dinv": dinv_t[k],
            "idxp": idx_pack[k],
            "selp": sel_pack[k],
            "ident": ident,
        }
        for k in range(NCORES)
    ]
    return meta, in_maps


def _build_program(meta):
    from concourse import bass, bacc, mybir
    import concourse.tile as tile

    q_rows = meta["q_rows"]
    chunk_rows = meta["chunk_rows"]
    S_wq = meta["S_wq"]
    sec0 = meta["sec0"]
    O_wq = meta["O_wq"]
    jmax = meta["jmax"]
    l16 = meta["l16"]
    plan = meta["plan"]
    mm_total = meta["mm_total"]
    selmax = meta["selmax"]
    ntiles_b = meta["ntiles_b"]

    f32 = mybir.dt.float32
    bf16 = mybir.dt.bfloat16
    fp8 = mybir.dt.float8e4

    nc = bacc.Bacc(num_swdge_queues=4)
    xt_in = nc.declare_dram_parameter("xt", [P, NBP], bf16, isOutput=False)
    w_in = nc.declare_dram_parameter("w_in", [P, P], bf16, isOutput=False)
    bias_in = nc.declare_dram_parameter("bias", [P, P], f32, isOutput=False)
    dinv_in = nc.declare_dram_parameter("dinv", [P, BLOCKS], f32, isOutput=False)
    idx_in = nc.declare_dram_parameter("idxp", [P, l16], mybir.dt.int16, isOutput=False)
    sel_in = nc.declare_dram_parameter("selp", [P, mm_total * P], fp8, isOutput=False)
    out_ext = nc.declare_dram_parameter("out", [NBP, P], f32, isOutput=True)

    hq = [nc.dram_tensor(f"h_q{q}", [q_rows[q], P], bf16) for q in range(4)]
    gq = [
        nc.dram_tensor(f"g_q{q}", [chunk_rows[q], P], bf16, addr_space="Shared")
        for q in range(4)
    ]

    # quarter row ranges within the shard, for gl (self-loop) loads
    q_off_r = [0]
    for q in range(3):
        q_off_r.append(q_off_r[-1] + q_rows[q])

    with tile.TileContext(nc) as tc:
        with (
            tc.tile_pool(name="const", bufs=1) as cpool,
            tc.tile_pool(name="work", bufs=4) as wpool,
            tc.tile_pool(name="msgp", bufs=2) as mpool,
            tc.tile_pool(name="selp", bufs=8) as spool,
            tc.tile_pool(name="glp", bufs=2) as gpool,
            tc.tile_pool(name="outp", bufs=2) as opool,
            tc.tile_pool(name="psA", bufs=2, space="PSUM") as psA,
            tc.tile_pool(name="psB", bufs=4, space="PSUM") as psB,
        ):
            # constants / metadata
            w_sb = cpool.tile([P, P], bf16, tag="w")
            nc.sync.dma_start(out=w_sb[:], in_=w_in[:])
            bias_sb = cpool.tile([P, P], f32, tag="bias")
            nc.sync.dma_start(out=bias_sb[:], in_=bias_in[:])
            dinv_sb = cpool.tile([P, BLOCKS], f32, tag="dinv")
            nc.sync.dma_start(out=dinv_sb[:], in_=dinv_in[:])
            idx_sb = cpool.tile([P, l16], mybir.dt.int16, tag="idx")
            for i in range(4):
                s = l16 // 4
                e = l16 if i == 3 else (i + 1) * s
                nc.sync.dma_start(out=idx_sb[:, i * s : e], in_=idx_in[:, i * s : e])


            # zero the two msg buffers once (pad slots are never gathered;
            # stale SBUF bits could be NaN in bf16 and 0*NaN would poison
            # the segment-sum matmuls)
            for _ in range(2):
                mz = mpool.tile([P, jmax, P], bf16, tag="msg")
                nc.vector.memset(mz[:], 0.0)

            # ---- phase A: h = (x @ W) * dinv, written per quarter, then
            # AllGather that quarter into the shared table chunk.
            for q in range(4):
                nblk = Q_BLKS[q]
                npc = -(-nblk // 6)
                lo = nblk // npc
                pieces = [lo + 1] * (nblk - lo * npc) + [lo] * (npc * (lo + 1) - nblk)
                assert sum(pieces) == nblk and max(pieces) <= 6
                loff = 0
                for pb in pieces:
                    gb0 = Q_OFF_B[q] + loff // P
                    xt_t = wpool.tile([P, 6 * P], bf16, tag="xt")
                    nc.sync.dma_start(
                        out=xt_t[:, : pb * P],
                        in_=xt_in[:, q_off_r[q] + loff : q_off_r[q] + loff + pb * P],
                    )
                    hbig = wpool.tile([P, 6, P], bf16, tag="hbig")
                    for j in range(pb):
                        ph = psA.tile([P, P], f32, tag="ph")
                        nc.tensor.matmul(
                            out=ph[:],
                            lhsT=xt_t[:, j * P : (j + 1) * P],
                            rhs=w_sb[:],
                            start=True,
                            stop=True,
                        )
                        gb = gb0 + j
                        nc.vector.tensor_scalar(
                            out=hbig[:, j, :],
                            in0=ph[:],
                            scalar1=dinv_sb[:, gb : gb + 1],
                            scalar2=None,
                            op0=mybir.AluOpType.mult,
                        )
                    nc.sync.dma_start(
                        out=hq[q][loff : loff + pb * P, :].rearrange(
                            "(j p) f -> p j f", p=P
                        ),
                        in_=hbig[:, :pb, :],
                    )
                    loff += pb * P
                nc.gpsimd.collective_compute(
                    "AllGather",
                    mybir.AluOpType.bypass,
                    replica_groups=[list(range(NCORES))],
                    ins=[hq[q][:]],
                    outs=[gq[q][:]],
                )

            # ---- phase B
            for w in range(NW):
                msg = mpool.tile([P, jmax, P], bf16, tag="msg")
                sels = [None] * 4
                for q in range(4):
                    swq = int(S_wq[w, q])
                    if swq == 0:
                        continue
                    s0 = int(sec0[w, q])
                    nc.gpsimd.dma_gather(
                        out_ap=msg[:, int(O_wq[w, q]) : int(O_wq[w, q]) + swq, :],
                        in_ap=gq[q][:],
                        idxs_ap=idx_sb[:, s0 * 8 : (s0 + swq) * 8],
                        num_idxs=swq * P,
                        num_idxs_reg=swq * P,
                        elem_size=P,
                        single_packet=False,
                        queue_num=q,
                    )
                    nmm = len(plan[w][q])
                    if nmm:
                        selw = spool.tile([P, selmax * P], fp8, tag="selw")
                        c0 = int(meta["selcol0"][w, q])
                        nc.scalar.dma_start(
                            out=selw[:, : nmm * P],
                            in_=sel_in[:, c0 * P : (c0 + nmm) * P],
                        )
                        sels[q] = selw

                # self-loop rows for this window (may straddle quarters)
                gl = gpool.tile([P, WBLK, P], bf16, tag="gl")
                r0, r1 = w * WBLK * P, (w + 1) * WBLK * P
                for q in range(4):
                    a = max(r0, q_off_r[q])
                    bnd = q_off_r[q] + q_rows[q]
                    bq = min(r1, bnd)
                    if a >= bq:
                        continue
                    j0 = (a - r0) // P
                    j1 = (bq - r0) // P
                    nc.sync.dma_start(
                        out=gl[:, j0:j1, :],
                        in_=hq[q][a - q_off_r[q] : bq - q_off_r[q], :].rearrange(
                            "(j p) f -> p j f", p=P
                        ),
                    )

                tmp_w = opool.tile([P, WBLK, P], f32, tag="tmpw")
                osb_w = opool.tile([P, WBLK, P], f32, tag="osbw")
                for j, bb in enumerate(range(w * WBLK, (w + 1) * WBLK)):
                    nt = int(ntiles_b[bb])
                    acc = psB.tile([P, P], f32, tag="acc")
                    ti = 0
                    for q in range(4):
                        base = int(meta["selcol0"][w, q])
                        for (b2, t, col) in plan[w][q]:
                            if b2 != bb:
                                continue
                            nc.tensor.matmul(
                                out=acc[:],
                                lhsT=sels[q][:, (col - base) * P : (col - base + 1) * P],
                                rhs=msg[:, int(O_wq[w, q]) + t, :],
                                start=(ti == 0),
                                stop=(ti == nt - 1),
                            )
                            ti += 1
                    assert ti == nt
                    # epilogue: (acc + h_own) * dinv_dst + bias
                    nc.vector.tensor_tensor(
                        out=tmp_w[:, j, :],
                        in0=acc[:],
                        in1=gl[:, j, :],
                        op=mybir.AluOpType.add,
                    )
                    nc.scalar.activation(
                        out=osb_w[:, j, :],
                        in_=tmp_w[:, j, :],
                        func=mybir.ActivationFunctionType.Copy,
                        scale=dinv_sb[:, bb : bb + 1],
                    )
                    nc.vector.tensor_tensor(
                        out=osb_w[:, j, :],
                        in0=osb_w[:, j, :],
                        in1=bias_sb[:],
                        op=mybir.AluOpType.add,
                    )
                nc.sync.dma_start(
                    out=out_ext[w * WBLK * P : (w + 1) * WBLK * P, :].rearrange(
                        "(j p) f -> p j f", p=P
                    ),
                    in_=osb_w[:],
                )

    nc.finalize()
    return nc


def _run(inputs, trace=False, trace_cores=None):
    from concourse.bass_utils import run_bass_kernel_spmd

    meta, in_maps = _pack(**inputs)
    nc = _build_program(meta)
    res = run_bass_kernel_spmd(
        nc,
        in_maps,
        list(range(NCORES)),
        trace=trace,
        trace_cores=trace_cores,
    )
    out = np.empty((N_NODES, P), np.float32)
    for k in range(NCORES):
        out[k * NB : (k + 1) * NB] = np.asarray(res.results[k]["out"])[:NB]
    return out, res


def kernel(x, edge_index, weight, b):
    out, _ = _run(dict(x=x, edge_index=edge_index, weight=weight, b=b))
    return out


if __name__ == "__main__":
    rng = np.random.default_rng(0)
    n, e = 100000, 1600000
    x = rng.standard_normal((n, P), dtype=np.float32)
    ei = rng.integers(0, n, (2, e)).astype(np.int64)
    w = (rng.standard_normal((P, P)) / np.sqrt(P)).astype(np.float32)
    bb = (rng.standard_normal(P) * 0.02).astype(np.float32)
    out = kernel(x, ei, w, bb)
    print("out", out.shape, out.dtype)
